# revision 68
# baseline (speedup 1.0000x reference)
"""Trainium2 Bass kernel for nn_AttentionBlock (B=4, H=W=64, C=64, GROUPS=32).

Math (reference):
    hn = GroupNorm(x; gamma, beta, 32 groups, eps=1e-3)
    q = hn@wq+bq ; k = hn@wk+bk ; v = hn@wv+bv
    att = softmax(q k^T / 8) over the 4096 spatial positions
    out = x + (att @ v) @ wo + bo

Sharding: data-parallel, 2 cores per batch image, each core owns 2048 of the
4096 queries but holds the full key/value set for its batch. No collectives.

Per-core pipeline (fully fused on one NeuronCore):
  - xT [C=64, S=4096] arrives pre-transposed in bf16 (host does the cheap
    numpy transpose+cast), so channel-contraction matmuls need no on-chip
    transposes. x_q keeps the core's own query rows in fp32 for the residual.
  - GroupNorm stats via bn_stats/bn_aggr per channel on DVE, then tiny 0/1
    matmuls pair-combine channels into groups and expand back. The GN affine
    folds into the projection weights: W~ = diag(scale_c)@W, b~ = gnbias@W + b.
  - k-bias is dropped: it shifts each query's scores by a constant, which
    softmax cancels exactly.
  - Scores are computed transposed, ST[t, s] (keys on partitions), so exp(ST)
    feeds the att@v matmul directly as the moving operand - the attention
    matrix is never transposed. Score matmuls have K=64, so two key-chunks run
    CONCURRENTLY on the two halves of the PE array (row-tiling): chunk p rides
    rows 0:63 and chunk 16+p rides rows 64:127 (kT stores each group on its
    own partition half; qT carries every column on both halves).
  - Softmax is max-free: |score| <= ~3 for unit-normal inputs so exp cannot
    overflow, and softmax(x) == softmax(x - max) exactly.
  - exp() runs one ACT instruction per chunk-pair over a 2-bank PSUM tile to
    amortize the ~352-cycle activation pipeline latency.
  - v gets an appended ones-column so att@v also accumulates the softmax
    denominator l[s]. att@v is split into two K=64 halves accumulating into
    two PSUM banks (summed by one DVE add at stripe end): the halves run on
    opposite array halves, letting LDWEIGHTS overlap in-flight matmuls.
  - The output projection runs on the unnormalized accumulator ((O/l)@wo ==
    (O@wo)/l), with an extra wo column passing l through; one reciprocal +
    fused multiply-add applies softmax normalization, residual and bo.
"""

import numpy as np
import ml_dtypes

import concourse.tile as tile
from concourse import bacc, mybir
from concourse.bass_utils import run_bass_kernel_spmd

F32 = mybir.dt.float32
BF16 = mybir.dt.bfloat16
U16 = mybir.dt.uint16
AF = mybir.ActivationFunctionType
ALU = mybir.AluOpType

# DVE bit-trick exp (Schraudolph in bf16): u16 = (uint16)(s*EXP_A + EXP_B);
# reinterpreting those bits as bf16 gives exp(s/8) with <=3.4% rel error
# (mean +0.2%), which softmax-averaging shrinks well below the tolerance.
# This lets the Vector engine absorb ~44% of the exp stream that otherwise
# serializes on the Scalar engine's activation LUT (1 elem/cycle/lane).
EXP_A = 0.125 * 1.4426950408889634 * 128.0   # scale * log2(e) * 2^7
EXP_B = 16256.0 - 5.5                        # 127*2^7 - c  (c tuned numerically)

B, H, W, C = 4, 64, 64, 64
S = H * W            # 4096 spatial positions per image
SQ = S // 2          # 2048 queries per core
EPS = 1e-3
N_CHUNK = S // 128   # 32 key chunks
NQ = SQ // 128       # 16 query chunks
N_STRIPE = SQ // 512  # 4 query stripes
SCALE = float(C) ** -0.5  # 0.125


def build_kernel():
    nc = bacc.Bacc("TRN2", target_bir_lowering=False, debug=False)

    xT_d = nc.dram_tensor("xT", [2 * C, S], BF16, kind="ExternalInput")
    x_q = nc.dram_tensor("x_q", [SQ, C], F32, kind="ExternalInput")
    # host-packed parameter blocks (pure layout prep: stacking/casting the
    # tiny 1x1-conv weights) - one DMA each instead of ~18 small DMAs whose
    # serial ~650ns descriptor-issue cost used to gate the whole preamble
    pk128 = nc.dram_tensor("pk128", [128, 259], F32, kind="ExternalInput")
    pk65 = nc.dram_tensor("pk65", [65, 129], F32, kind="ExternalInput")
    pkb = nc.dram_tensor("pkb", [C, 65], BF16, kind="ExternalInput")
    bo_d = nc.dram_tensor("bo", [C], F32, kind="ExternalInput")
    out_d = nc.dram_tensor("out", [SQ, C], F32, kind="ExternalOutput")

    with tile.TileContext(nc) as tc:
        _emit(nc, tc, xT_d.ap(), x_q.ap(), pk128.ap(), pk65.ap(), pkb.ap(),
              bo_d.ap(), out_d.ap())
    nc.compile()
    return nc


def _emit(nc, tc, xT_d, x_q, pk128_d, pk65_d, pkb_d, bo_d, out_d):
    from contextlib import ExitStack

    ctx = ExitStack()
    with ctx:
        const = ctx.enter_context(tc.tile_pool(name="const", bufs=1))
        big = ctx.enter_context(tc.tile_pool(name="big", bufs=1))
        tiny = ctx.enter_context(tc.tile_pool(name="tiny", bufs=1))

        # ---- big input DMAs first (sync/HWDGE ring), chunked so dependents
        # ---- can start early. Partitions 64:127 hold a copy ROTATED by 2048
        # ---- columns (xT[64:128, c] = x[:, (c+2048) % S]): row-tiled matmuls
        # ---- just index hi-half columns with a -2048 shift, and GroupNorm
        # ---- stats get a [128, 2048] view covering all 4096 positions with
        # ---- every partition lane active (2.4x faster bn_stats).
        # The host ships BOTH partition halves (hi half pre-rotated), so the
        # four chunk DMAs are independent - no SBUF->SBUF mirror chain.
        xT = big.tile([128, S], BF16)
        eng = [nc.sync, nc.scalar, nc.gpsimd, nc.sync,
               nc.scalar, nc.gpsimd, nc.sync, nc.scalar]
        for i in range(8):
            eng[i].dma_start(out=xT[:, 512 * i:512 * (i + 1)],
                             in_=xT_d[:, 512 * i:512 * (i + 1)])

        # ---- PE warmup spin: the PE clock sits at 1.2 GHz until ~3.4us of
        # sustained activity un-throttles it to 2.4 GHz. Dummy matmuls on a
        # zeroed tile during the DMA/stats wait warm it up so the projections
        # and the first attention stripe run at full rate. Memsets run on the
        # (otherwise idle) Vector engine so the gpsimd queue issues its xT
        # DMAs immediately.
        wsp = const.tile([128, 192], BF16)
        nc.vector.memset(wsp, 0.0)

        zbias = const.tile([128, 1], F32)
        nc.vector.memset(zbias, 0.0)
        # exp is the only ACT table set this kernel uses (rsqrt is done with a
        # Newton iteration on DVE); preload it while waiting on input DMAs.
        scratch1 = const.tile([1, 1], F32)
        nc.scalar.activation(scratch1, zbias[0:1, :], AF.Exp, bias=0.0, scale=1.0)

        # pair matrices: p64h[c,g] = 0.25 iff (c%64)//2 == g (both partition
        # halves, each holding one half-image under the rotated-mirror
        # layout); p32x64[g,c] = 1 iff c//2 == g
        p64h = const.tile([128, 32], BF16)
        nc.vector.memset(p64h[0:64, :], 0.25)
        nc.gpsimd.affine_select(out=p64h[0:64, :], in_=p64h[0:64, :],
                                compare_op=ALU.is_ge,
                                fill=0.0, base=0, pattern=[[-2, 32]],
                                channel_multiplier=1)
        nc.gpsimd.affine_select(out=p64h[0:64, :], in_=p64h[0:64, :],
                                compare_op=ALU.is_ge,
                                fill=0.0, base=1, pattern=[[2, 32]],
                                channel_multiplier=-1)
        nc.gpsimd.dma_start(out=p64h[64:128, :], in_=p64h[0:64, :])
        p32x64 = const.tile([32, 64], BF16)
        nc.vector.memset(p32x64, 1.0)
        nc.gpsimd.affine_select(out=p32x64, in_=p32x64, compare_op=ALU.is_ge,
                                fill=0.0, base=0, pattern=[[1, 64]],
                                channel_multiplier=-2)
        nc.gpsimd.affine_select(out=p32x64, in_=p32x64, compare_op=ALU.is_ge,
                                fill=0.0, base=1, pattern=[[-1, 64]],
                                channel_multiplier=2)

        # ---- PSUM pools (8 banks: st 3x[128,1024]=6, acc 2x1) ----
        # Three score buffers break the serial chain exp(p) -> scores(p+2) ->
        # exp(p+2): with bufs=3 a score matmul waits on the exp THREE pairs
        # back (the other engine's, long done), so the two exp engines stream
        # at their native rate.
        st_ps = ctx.enter_context(tc.tile_pool(name="st_ps", bufs=3, space="PSUM"))
        aux_ps = ctx.enter_context(tc.tile_pool(name="aux_ps", bufs=2, space="PSUM"))

        # PE warmup spin (see wsp note above): no-op matmuls bridge the idle
        # window between kernel start and the first real matmuls.
        warm = st_ps.tile([128, 1024], F32, tag="st")
        for _ in range(48):
            nc.tensor.matmul(warm[0:64, 0:128], lhsT=wsp[:, 0:64],
                             rhs=wsp[:, 64:192], start=True, stop=True)

        # ---- GroupNorm stats on DVE: all 128 partitions active; partition
        # ---- p>=64 carries channel p-64 over the second half-image ----
        bstats = tiny.tile([128, 4, 6], F32)
        for i in range(4):
            nc.vector.bn_stats(bstats[:, i, :], xT[:, 512 * i:512 * (i + 1)])
        mv = tiny.tile([128, 2], F32)
        nc.vector.bn_aggr(mv, bstats)
        packed128 = tiny.tile([128, 2], BF16)     # [mean, E[x^2]] per (c,half)
        nc.vector.tensor_copy(packed128[:, 0:1], mv[:, 0:1])
        nc.vector.scalar_tensor_tensor(out=packed128[:, 1:2], in0=mv[:, 0:1],
                                       scalar=mv[:, 0:1], in1=mv[:, 1:2],
                                       op0=ALU.mult, op1=ALU.add)

        # ---- host-packed params (emitted AFTER the stats: the per-queue DMA
        # ---- semaphore waits coalesce in program order, so anything emitted
        # ---- earlier on these rings would gate bn_stats) ----
        pk_sb = const.tile([128, 259], F32)
        nc.scalar.dma_start(out=pk_sb, in_=pk128_d)
        wk_sb = pk_sb[:, 0:64]        # [wk; wk]
        wq_sb = pk_sb[:, 64:128]      # [wq; wq]
        wv_sb = pk_sb[:, 128:192]     # [wv; wv]
        gamma_col = pk_sb[:, 192:193]
        beta_col = pk_sb[0:64, 193:194]
        wo_sb = pk_sb[0:64, 194:258]
        pk65_sb = const.tile([65, 129], F32)
        nc.scalar.dma_start(out=pk65_sb, in_=pk65_d)
        wq_aug = pk65_sb[:, 0:64]     # [Wq ; bq]
        wv_aug = pk65_sb[:, 64:129]   # [Wv ; bv] plus e64 column
        # wo_aug = [wo ; bvo+bo] plus e64 column that passes l through. Row
        # 64 multiplies the l-row of the accumulator, so after the division
        # by l it contributes the constant row (bv_total @ wo) + bo - the v-
        # and output-bias applied without materializing them per-position.
        wo_aug = const.tile([65, 65], BF16)
        nc.scalar.dma_start(out=wo_aug[0:64, :], in_=pkb_d)
        nc.vector.memset(wo_aug[64:65, 64:65], 1.0)
        bo_row = const.tile([1, 64], F32)
        nc.scalar.dma_start(out=bo_row, in_=bo_d.rearrange("(o c) -> o c", o=1))

        gpair = aux_ps.tile([32, 2], F32, tag="aux")  # group [mean, E[x^2]]
        nc.tensor.matmul(gpair, lhsT=p64h, rhs=packed128)
        gm = tiny.tile([32, 2], F32)
        nc.vector.tensor_copy(gm, gpair)
        var = tiny.tile([32, 1], F32)
        nc.vector.tensor_mul(var, gm[:, 0:1], gm[:, 0:1])
        nc.vector.tensor_sub(var, gm[:, 1:2], var)
        nc.vector.tensor_scalar_add(var, var, EPS)
        # rstd = rsqrt(var) entirely on DVE: quake-style bit seed + 1 Newton
        # step (seed err ~3.4% -> ~0.2% after the step; GroupNorm scale is
        # far inside the tolerance) - keeps the scalar engine's activation
        # tables untouched for exp, and the serial chain short.
        U32 = mybir.dt.uint32
        magic = tiny.tile([32, 1], U32)
        nc.vector.memset(magic, 0x5f3759df)
        packed32 = tiny.tile([32, 2], BF16)       # [rstd_g | mean_g]
        nc.vector.tensor_copy(packed32[:, 1:2], gm[:, 0:1])
        ybits = tiny.tile([32, 1], U32)
        nc.vector.tensor_scalar(out=ybits, in0=var.bitcast(U32), scalar1=1,
                                scalar2=None, op0=ALU.logical_shift_right)
        nc.vector.tensor_sub(ybits, magic, ybits)
        y = ybits.bitcast(F32)
        c15 = tiny.tile([32, 1], F32)
        nc.vector.memset(c15, 1.5)
        nhv = tiny.tile([32, 1], F32)
        nc.vector.tensor_scalar_mul(nhv, var, -0.5)
        t1 = tiny.tile([32, 1], F32)
        nc.vector.tensor_mul(t1, y, y)
        nc.vector.scalar_tensor_tensor(out=t1, in0=t1, scalar=nhv, in1=c15,
                                       op0=ALU.mult, op1=ALU.add)
        nc.vector.tensor_mul(packed32[:, 0:1], y, t1)
        rstd = packed32[:, 0:1]
        chan = aux_ps.tile([128, 2], F32, tag="aux")  # expand groups->channels,
        nc.tensor.matmul(chan[0:64, :], lhsT=p32x64, rhs=packed32)  # both halves
        nc.tensor.matmul(chan[64:128, :], lhsT=p32x64, rhs=packed32,
                         tile_position=(0, 64))
        scale_col = tiny.tile([128, 1], F32)      # rstd_g * gamma_c (mirrored)
        nc.vector.tensor_mul(scale_col, chan[:, 0:1], gamma_col)
        gnbias = tiny.tile([65, 1], F32)          # beta - mean*scale, aug 1
        nc.vector.tensor_mul(gnbias[0:64, :], chan[0:64, 1:2], scale_col[0:64, :])
        nc.vector.tensor_sub(gnbias[0:64, :], beta_col, gnbias[0:64, :])
        nc.vector.memset(gnbias[64:65, :], 1.0)

        # ---- fold GN into projection weights (both halves in one op); on
        # ---- ACT (Copy with per-partition scale AP) in parallel with the
        # ---- DVE's serial gnbias chain ----
        wv_sc = tiny.tile([128, 64], BF16)
        nc.scalar.activation(wv_sc, wv_sb, AF.Copy, scale=scale_col)
        wk_sc = tiny.tile([128, 64], BF16)
        nc.scalar.activation(wk_sc, wk_sb, AF.Copy, scale=scale_col)
        wq_sc = tiny.tile([128, 64], BF16)
        nc.scalar.activation(wq_sc, wq_sb, AF.Copy, scale=scale_col)

        bqp = aux_ps.tile([128, 1], F32, tag="aux")  # total q bias, both halves
        nc.tensor.matmul(bqp[0:64, :], lhsT=wq_aug, rhs=gnbias)
        nc.tensor.matmul(bqp[64:128, :], lhsT=wq_aug, rhs=gnbias,
                         tile_position=(0, 64))
        bq_col = tiny.tile([128, 1], F32)
        nc.vector.tensor_copy(bq_col, bqp)
        # bvo row for wo_aug: bvo = (gnbias@Wv + bv) @ wo, bounced through HBM
        # to land on partition 64 (engines are lane-locked; DMA is not). This
        # only gates the first output projection, well off the critical path.
        bvcp = aux_ps.tile([65, 1], F32, tag="aux")
        nc.tensor.matmul(bvcp, lhsT=wv_aug, rhs=gnbias)
        bv_col = tiny.tile([64, 1], F32)
        nc.vector.tensor_copy(bv_col, bvcp[0:64, :])
        bvop = aux_ps.tile([1, 64], F32, tag="aux")
        nc.tensor.matmul(bvop, lhsT=bv_col, rhs=wo_sb)
        # fold the OUTPUT bias bo in as well: row 64 of wo_aug contributes
        # (bvo+bo)*l to the pre-division accumulator, i.e. bvo+bo after the
        # division - so the epilogue residual add uses x directly.
        bvo_row = tiny.tile([1, 64], F32)
        nc.vector.tensor_add(bvo_row, bvop, bo_row)
        bvo_stage = nc.dram_tensor("bvo_stage", [64], F32).ap()
        nc.sync.dma_start(out=bvo_stage.rearrange("(o c) -> o c", o=1), in_=bvo_row)
        nc.gpsimd.dma_start(out=wo_aug[64:65, 0:64],
                            in_=bvo_stage.rearrange("(o c) -> o c", o=1))

        # ---- projections ----
        # Score matmuls pair key-chunk p (array rows 0:63) with chunk 16+p
        # (rows 64:127) - attention is order-invariant over keys - so kT keeps
        # chunks 0:15 on partitions 0:63 and chunks 16:31 on 64:127, and each
        # projection block lands directly on its half (output half chosen by
        # the tile_position column). qT needs every column on BOTH halves:
        # 4 tile-position variants cover the (stripe, half) grid pairwise-
        # concurrently. No SBUF mirror DMAs anywhere.
        kT = big.tile([128, SQ], BF16)
        qT = big.tile([128, SQ], BF16)

        # k/q projected in quad-groups: lo and hi array halves fill the two
        # partition halves of one [128,1024] tile (bank-staggered waves so
        # concurrent row-tiles never drain into the same bank), then ONE
        # full-width copy/bias-add moves both halves. kT's hi half holds
        # chunks 16:31 (see pairing note above); qT needs both halves of
        # every column, which this layout produces naturally.
        def kq_quad(dst, w_sc, lo_cols, hi_cols, bias, on_act):
            g = st_ps.tile([128, 1024], F32, tag="st")
            nc.tensor.matmul(g[0:64, 0:512], lhsT=w_sc[0:64, :],
                             rhs=xT[0:64, lo_cols:lo_cols + 512],
                             tile_position=(0, 0))
            nc.tensor.matmul(g[64:128, 512:1024], lhsT=w_sc[64:128, :],
                             rhs=xT[64:128, hi_cols + 512:hi_cols + 1024],
                             tile_position=(64, 64))
            nc.tensor.matmul(g[0:64, 512:1024], lhsT=w_sc[0:64, :],
                             rhs=xT[0:64, lo_cols + 512:lo_cols + 1024],
                             tile_position=(0, 0))
            nc.tensor.matmul(g[64:128, 0:512], lhsT=w_sc[64:128, :],
                             rhs=xT[64:128, hi_cols:hi_cols + 512],
                             tile_position=(64, 64))
            # drains alternate between ACT and DVE so neither engine builds a
            # pre-loop backlog (ACT applies a per-partition bias for free via
            # Identity+bias AP)
            if bias is None:
                if on_act:
                    nc.scalar.copy(out=dst, in_=g)
                else:
                    nc.vector.tensor_copy(dst, g)
            elif on_act:
                nc.scalar.activation(dst, g, AF.Identity, bias=bias)
            else:
                nc.vector.tensor_scalar_add(dst, g, bias)

        # v in natural [t, c] layout; groups of 4 chunks {p, 8+p, 16+p, 24+p}
        # share one PSUM bank (quarter slices) and drain with one strided copy.
        # Chunks p,8+p ride array rows 0:63, 16+p,24+p rows 64:127 so the two
        # sub-pairs run concurrently. Column 64 = ones via one strided memset.
        # Emitted BEFORE the kq quads: the first att@v (3 pairs into the
        # loop) needs v chunks 0 and 16, while the first scores only need the
        # kT quad - v-first keeps the loop's start unblocked.
        v_big = big.tile([128, N_CHUNK, 65], BF16)
        nc.vector.memset(v_big[:, :, 64:65], 1.0)
        # view: chunk (a*16 + b*8 + p) -> [p_, a, b, p, c]; lo-rows compute the
        # a=0 chunks into bank 0 of a 2-bank tile, hi-rows the a=1 chunks into
        # bank 1 (concurrent row-tiles must drain into distinct banks).
        v4 = v_big.rearrange("q (a b g) c -> q a b g c", a=2, b=2)

        def v_group(p):
            vga = aux_ps.tile([128, 2, 64], F32, tag="aux")
            vgb = aux_ps.tile([128, 2, 64], F32, tag="aux")
            for a, vg in ((0, vga), (1, vgb)):
                half = slice(64, 128) if a else slice(0, 64)
                tp = (64, 0) if a else (0, 0)
                for b in range(2):
                    ch = a * 16 + b * 8 + p
                    # hi-half (a=1) columns shift -2048 (rotated mirror)
                    c0 = 128 * ch - 2048 * a
                    nc.tensor.matmul(vg[:, b, :],
                                     lhsT=xT[half, c0:c0 + 128],
                                     rhs=wv_sc[half, :], tile_position=tp)
            # drains split across ACT/DVE so neither builds a pre-loop backlog
            nc.scalar.copy(out=v4[:, 0, :, p, 0:64], in_=vga)
            nc.vector.tensor_copy(v4[:, 1, :, p, 0:64], vgb)

        for p in range(8):
            v_group(p)

        # hi-half column indices are shifted -2048 under the rotated mirror
        kq_quad(kT[:, 0:1024], wk_sc, 0, 0, None, True)
        kq_quad(qT[:, 0:1024], wq_sc, 0, 2048, bq_col, True)
        kq_quad(kT[:, 1024:2048], wk_sc, 1024, 1024, None, False)
        kq_quad(qT[:, 1024:2048], wq_sc, 1024, 3072, bq_col, False)

        # ---- residual rows (needed only by epilogues; bo is folded into the
        # ---- wo_aug l-row, so x is used directly) ----
        xq_sb = big.tile([128, NQ, 64], F32)
        xq_r = x_q.rearrange("(m p) c -> p m c", p=128)
        nc.sync.dma_start(out=xq_sb[:, 0:8, :], in_=xq_r[:, 0:8, :])
        nc.scalar.dma_start(out=xq_sb[:, 8:16, :], in_=xq_r[:, 8:16, :])

        # ---- main attention loop ----
        # Pairs of key chunks: the two K=64 score matmuls run concurrently on
        # the two row-halves of the PE array into the two banks of one PSUM
        # tile; exp covers both in one instruction, alternating between the
        # Scalar engine's LUT exp and the Vector engine's bit-trick exp so the
        # two engines stream concurrently. att@v for chunk c is ONE full-K=128
        # matmul accumulating into a single PSUM bank (no merge needed), and
        # trails the score/exp stream by LAG pairs: the PE issues in order, so
        # by the time it reaches att@v(p), exp(p) - issued LAG pairs earlier -
        # is already complete and the queue never stalls.
        p_pool = ctx.enter_context(tc.tile_pool(name="p_pool", bufs=6))
        ep_pool = ctx.enter_context(tc.tile_pool(name="ep_pool", bufs=3))
        N_PAIR = N_CHUNK // 2
        LAG = 3
        # 7 odd pairs on DVE, 9 on ACT: balances ACT's 1.11us/tile + drain-lo
        # vs DVE's 1.22us/tile + drain-hi + epilogue work.
        DVE_PAIRS = {1, 3, 5, 7, 9, 11, 13}

        def emit_o(io, ot_lo, ot_hi, pt, first, last):
            # K=64 halves on opposite array row-halves run concurrently and
            # let LDWEIGHTS overlap in-flight matmuls (different row groups);
            # each half accumulates into its own bank. The banks are never
            # summed directly: the output projection is linear, so the
            # epilogue projects each half separately and accumulates in PSUM.
            nc.tensor.matmul(ot_lo, lhsT=v_big[0:64, io, :], rhs=pt[0:64, :],
                             tile_position=(0, 0), start=first, stop=last)
            nc.tensor.matmul(ot_hi, lhsT=v_big[64:128, io, :], rhs=pt[64:128, :],
                             tile_position=(64, 0), start=first, stop=last)

        def make_epilogue(j, lo_sb, hi_sb, tail=False):
            def epi():
                res = ep_pool.tile([128, 4, 64], F32, tag="res", bufs=2)
                rl = ep_pool.tile([128, 4], F32, tag="rl")
                # borrows an st-pool slot mid-stripe; the slot's previous
                # score tile is consumed by then and its next user comes ~3
                # pairs later, so the rotation never stalls.
                opq = st_ps.tile([128, 4, 65], F32, tag="st")
                for m in range(4):
                    cs = slice(128 * m, 128 * (m + 1))
                    nc.tensor.matmul(opq[:, m, :], lhsT=lo_sb[:, cs],
                                     rhs=wo_aug, start=True, stop=False)
                    nc.tensor.matmul(opq[:, m, :], lhsT=hi_sb[:, cs],
                                     rhs=wo_aug, start=False, stop=True)
                if tail:
                    # the last stripe is the critical exit path: normalize and
                    # ship each 128-row chunk on its own DMA ring so the
                    # ~650ns per-issue cost doesn't serialize
                    for m in range(4):
                        nc.vector.reciprocal(rl[:, m:m + 1], opq[:, m, 64:65])
                        nc.vector.scalar_tensor_tensor(
                            out=res[:, m, :], in0=opq[:, m, 0:64],
                            scalar=rl[:, m:m + 1], in1=xq_sb[:, 4 * j + m, :],
                            op0=ALU.mult, op1=ALU.add)
                        base = 512 * j + 128 * m
                        ring = [nc.sync, nc.scalar, nc.gpsimd, nc.sync][m]
                        ring.dma_start(out=out_d[base:base + 128, :],
                                       in_=res[:, m, :])
                else:
                    nc.vector.reciprocal(rl, opq[:, :, 64])
                    for m in range(4):
                        nc.vector.scalar_tensor_tensor(
                            out=res[:, m, :], in0=opq[:, m, 0:64],
                            scalar=rl[:, m:m + 1], in1=xq_sb[:, 4 * j + m, :],
                            op0=ALU.mult, op1=ALU.add)
                    nc.sync.dma_start(
                        out=out_d[512 * j:512 * (j + 1), :].rearrange("(m p) c -> p m c", p=128),
                        in_=res)
            return epi

        # Flat 64-pair pipeline: pair g = stripe g//16, pair-in-stripe g%16.
        # The att@v stream trails by LAG pairs and crosses stripe boundaries
        # without draining the pipe; each stripe's accumulator drains are
        # emitted right after its last att@v (landing ~2 pairs into the next
        # stripe on the exp engines' queues), and its epilogue 3 pairs after
        # that, so neither ever blocks the next stripe's exps.
        NG = N_STRIPE * N_PAIR
        pts = {}
        ots = {}
        lohi = {}
        for g in range(NG + LAG):
            if g < NG:
                j, p = divmod(g, N_PAIR)
                if p == 0:
                    ots[j] = (aux_ps.tile([65, 512], F32, tag="aux", name=f"ot_lo{j}"),
                              aux_ps.tile([65, 512], F32, tag="aux", name=f"ot_hi{j}"))
                st2 = st_ps.tile([128, 1024], F32, tag="st")
                nc.tensor.matmul(st2[:, 0:512],
                                 lhsT=kT[0:64, 128 * p:128 * (p + 1)],
                                 rhs=qT[0:64, 512 * j:512 * (j + 1)],
                                 tile_position=(0, 0))
                nc.tensor.matmul(st2[:, 512:1024],
                                 lhsT=kT[64:128, 128 * p:128 * (p + 1)],
                                 rhs=qT[64:128, 512 * j:512 * (j + 1)],
                                 tile_position=(64, 0))
                pt = p_pool.tile([128, 1024], BF16, tag="p")
                if p in DVE_PAIRS:
                    nc.vector.tensor_scalar(pt.bitcast(U16), st2, EXP_A,
                                            EXP_B, ALU.mult, ALU.add)
                else:
                    nc.scalar.activation(pt, st2, AF.Exp, bias=zbias,
                                         scale=SCALE)
                pts[g] = pt
            if g >= LAG:
                go = g - LAG
                jo, po = divmod(go, N_PAIR)
                pt = pts.pop(go)
                ot_lo, ot_hi = ots[jo]
                emit_o(po, ot_lo, ot_hi, pt[:, 0:512], po == 0, False)
                emit_o(16 + po, ot_lo, ot_hi, pt[:, 512:1024], False,
                       po == N_PAIR - 1)
                if po == N_PAIR - 1:
                    # drain the stripe's two accumulator banks (lo on Scalar,
                    # hi on Vector so neither becomes the bottleneck)
                    lo_sb = ep_pool.tile([65, 512], BF16, bufs=2, tag="lo_sb")
                    hi_sb = ep_pool.tile([65, 512], BF16, bufs=2, tag="hi_sb")
                    nc.scalar.copy(out=lo_sb, in_=ot_lo)
                    nc.vector.tensor_copy(hi_sb, ot_hi)
                    lohi[jo] = (lo_sb, hi_sb)
                    del ots[jo]
                if po == 2 and jo >= 1:
                    make_epilogue(jo - 1, *lohi.pop(jo - 1))()
        make_epilogue(N_STRIPE - 1, *lohi.pop(N_STRIPE - 1), tail=True)()


_NC_CACHE = {}


def _get_nc():
    if "nc" not in _NC_CACHE:
        _NC_CACHE["nc"] = build_kernel()
    return _NC_CACHE["nc"]


def build_in_maps(x, gamma, beta, wq, bq, wk, wv, bv, wo, bo):
    """Per-core NEFF input dicts plus (batch, rows) scatter info per core."""
    x = np.asarray(x, dtype=np.float32)
    gamma = np.asarray(gamma, np.float32)
    beta = np.asarray(beta, np.float32)
    wq = np.asarray(wq, np.float32)
    bq = np.asarray(bq, np.float32)
    wk = np.asarray(wk, np.float32)
    wv = np.asarray(wv, np.float32)
    bv = np.asarray(bv, np.float32)
    wo = np.asarray(wo, np.float32)
    bo = np.asarray(bo, np.float32)
    # pack the small weights into three layout blocks (see _emit)
    pk128 = np.zeros((128, 259), np.float32)
    pk128[0:64, 0:64] = wk
    pk128[64:128, 0:64] = wk
    pk128[0:64, 64:128] = wq
    pk128[64:128, 64:128] = wq
    pk128[0:64, 128:192] = wv
    pk128[64:128, 128:192] = wv
    pk128[0:64, 192] = gamma
    pk128[64:128, 192] = gamma
    pk128[0:64, 193] = beta
    pk128[0:64, 194:258] = wo
    pk65 = np.zeros((65, 129), np.float32)
    pk65[0:64, 0:64] = wq
    pk65[64, 0:64] = bq
    pk65[0:64, 64:128] = wv
    pk65[64, 64:128] = bv
    pk65[64, 128] = 1.0
    pkb = np.zeros((64, 65), np.float32)
    pkb[:, 0:64] = wo
    shared = {
        "pk128": pk128,
        "pk65": pk65,
        "pkb": pkb.astype(ml_dtypes.bfloat16),
        "bo": bo,
    }
    xf = x.reshape(B, S, C)
    in_maps = []
    scatter = []
    for core in range(8):
        b, h = core // 2, core % 2
        own = slice(h * SQ, (h + 1) * SQ)
        other = slice((1 - h) * SQ, (2 - h) * SQ)
        x_local = np.concatenate([xf[b][own], xf[b][other]], axis=0)
        # partitions 0:64 = channels x positions; 64:128 = the same rotated
        # by 2048 columns (see the kernel's rotated-mirror layout note)
        xt = np.ascontiguousarray(x_local.T).astype(ml_dtypes.bfloat16)
        xt2 = np.concatenate([xt, np.roll(xt, -SQ, axis=1)], axis=0)
        in_maps.append({
            "xT": np.ascontiguousarray(xt2),
            "x_q": np.ascontiguousarray(x_local[:SQ]),
            **shared,
        })
        scatter.append((b, np.arange(h * SQ, (h + 1) * SQ)))
    return in_maps, scatter


def _run(in_maps, scatter, **spmd_kwargs):
    nc = _get_nc()
    res = run_bass_kernel_spmd(nc, in_maps, core_ids=list(range(8)),
                               **spmd_kwargs)
    out = np.empty((B, S, C), np.float32)
    for core in range(8):
        b, rows = scatter[core]
        out[b][rows] = res.results[core]["out"]
    return out.reshape(B, H, W, C), res


def kernel(x, gamma, beta, wq, bq, wk, bk, wv, bv, wo, bo):
    # bk is provably a no-op: it shifts each query's scores by the constant
    # bk.q which softmax cancels, so it is not shipped to the device.
    in_maps, scatter = build_in_maps(x, gamma, beta, wq, bq, wk, wv, bv, wo, bo)
    out, _ = _run(in_maps, scatter)
    return out



# revision 71
# speedup vs baseline: 1.1797x; 1.1797x over previous
"""Trainium2 Bass kernel for nn_AttentionBlock (B=4, H=W=64, C=64, GROUPS=32).

Math (reference):
    hn = GroupNorm(x; gamma, beta, 32 groups, eps=1e-3)
    q = hn@wq+bq ; k = hn@wk+bk ; v = hn@wv+bv
    att = softmax(q k^T / 8) over the 4096 spatial positions
    out = x + (att @ v) @ wo + bo

Sharding: data-parallel, 2 cores per batch image, each core owns 2048 of the
4096 queries but holds the full key/value set for its batch. No collectives.

Per-core pipeline (fully fused on one NeuronCore):
  - xT [C=64, S=4096] arrives pre-transposed in bf16 (host does the cheap
    numpy transpose+cast), so channel-contraction matmuls need no on-chip
    transposes. x_q keeps the core's own query rows in fp32 for the residual.
  - GroupNorm stats via bn_stats/bn_aggr per channel on DVE, then tiny 0/1
    matmuls pair-combine channels into groups and expand back. The GN affine
    folds into the projection weights: W~ = diag(scale_c)@W, b~ = gnbias@W + b.
  - k-bias is dropped: it shifts each query's scores by a constant, which
    softmax cancels exactly.
  - Scores are computed transposed, ST[t, s] (keys on partitions), so exp(ST)
    feeds the att@v matmul directly as the moving operand - the attention
    matrix is never transposed. Score matmuls have K=64, so two key-chunks run
    CONCURRENTLY on the two halves of the PE array (row-tiling): chunk p rides
    rows 0:63 and chunk 16+p rides rows 64:127 (kT stores each group on its
    own partition half; qT carries every column on both halves).
  - Softmax is max-free: |score| <= ~3 for unit-normal inputs so exp cannot
    overflow, and softmax(x) == softmax(x - max) exactly.
  - exp() runs one ACT instruction per chunk-pair over a 2-bank PSUM tile to
    amortize the ~352-cycle activation pipeline latency.
  - v gets an appended ones-column so att@v also accumulates the softmax
    denominator l[s]. att@v is split into two K=64 halves accumulating into
    two PSUM banks (summed by one DVE add at stripe end): the halves run on
    opposite array halves, letting LDWEIGHTS overlap in-flight matmuls.
  - The output projection runs on the unnormalized accumulator ((O/l)@wo ==
    (O@wo)/l), with an extra wo column passing l through; one reciprocal +
    fused multiply-add applies softmax normalization, residual and bo.
"""

import numpy as np
import ml_dtypes

import concourse.tile as tile
from concourse import bacc, mybir
from concourse.bass_utils import run_bass_kernel_spmd

F32 = mybir.dt.float32
BF16 = mybir.dt.bfloat16
U16 = mybir.dt.uint16
AF = mybir.ActivationFunctionType
ALU = mybir.AluOpType

# DVE bit-trick exp (Schraudolph in bf16): u16 = (uint16)(s*EXP_A + EXP_B);
# reinterpreting those bits as bf16 gives exp(s/8) with <=3.4% rel error
# (mean +0.2%), which softmax-averaging shrinks well below the tolerance.
# This lets the Vector engine absorb ~44% of the exp stream that otherwise
# serializes on the Scalar engine's activation LUT (1 elem/cycle/lane).
EXP_A = 0.125 * 1.4426950408889634 * 128.0   # scale * log2(e) * 2^7
EXP_B = 16256.0 - 5.5                        # 127*2^7 - c  (c tuned numerically)

B, H, W, C = 4, 64, 64, 64
S = H * W            # 4096 spatial positions per image
SQ = S // 2          # 2048 queries per core
EPS = 1e-3
N_CHUNK = S // 128   # 32 key chunks
NQ = SQ // 128       # 16 query chunks
N_STRIPE = SQ // 512  # 4 query stripes
SCALE = float(C) ** -0.5  # 0.125


def build_kernel():
    nc = bacc.Bacc("TRN2", target_bir_lowering=False, debug=False)

    xT_d = nc.dram_tensor("xT", [2 * C, S], BF16, kind="ExternalInput")
    x_q = nc.dram_tensor("x_q", [SQ, C], F32, kind="ExternalInput")
    # host-packed parameter blocks (pure layout prep: stacking/casting the
    # tiny 1x1-conv weights) - one DMA each instead of ~18 small DMAs whose
    # serial ~650ns descriptor-issue cost used to gate the whole preamble
    pk128 = nc.dram_tensor("pk128", [128, 259], F32, kind="ExternalInput")
    pk65 = nc.dram_tensor("pk65", [65, 129], F32, kind="ExternalInput")
    pkb = nc.dram_tensor("pkb", [C, 65], BF16, kind="ExternalInput")
    bo_d = nc.dram_tensor("bo", [C], F32, kind="ExternalInput")
    out_d = nc.dram_tensor("out", [SQ, C], F32, kind="ExternalOutput")

    with tile.TileContext(nc) as tc:
        _emit(nc, tc, xT_d.ap(), x_q.ap(), pk128.ap(), pk65.ap(), pkb.ap(),
              bo_d.ap(), out_d.ap())
    nc.compile()
    return nc


def _emit(nc, tc, xT_d, x_q, pk128_d, pk65_d, pkb_d, bo_d, out_d):
    from contextlib import ExitStack

    ctx = ExitStack()
    with ctx:
        const = ctx.enter_context(tc.tile_pool(name="const", bufs=1))
        big = ctx.enter_context(tc.tile_pool(name="big", bufs=1))
        tiny = ctx.enter_context(tc.tile_pool(name="tiny", bufs=1))

        # ---- big input DMAs first (sync/HWDGE ring), chunked so dependents
        # ---- can start early. Partitions 64:127 hold a copy ROTATED by 2048
        # ---- columns (xT[64:128, c] = x[:, (c+2048) % S]): row-tiled matmuls
        # ---- just index hi-half columns with a -2048 shift, and GroupNorm
        # ---- stats get a [128, 2048] view covering all 4096 positions with
        # ---- every partition lane active (2.4x faster bn_stats).
        # The host ships BOTH partition halves (hi half pre-rotated), so the
        # four chunk DMAs are independent - no SBUF->SBUF mirror chain.
        xT = big.tile([128, S], BF16)
        eng = [nc.sync, nc.scalar, nc.gpsimd, nc.sync,
               nc.scalar, nc.gpsimd, nc.sync, nc.scalar]
        for i in range(8):
            eng[i].dma_start(out=xT[:, 512 * i:512 * (i + 1)],
                             in_=xT_d[:, 512 * i:512 * (i + 1)])

        # ---- PE warmup spin: the PE clock sits at 1.2 GHz until ~3.4us of
        # sustained activity un-throttles it to 2.4 GHz. Dummy matmuls on a
        # zeroed tile during the DMA/stats wait warm it up so the projections
        # and the first attention stripe run at full rate. Memsets run on the
        # (otherwise idle) Vector engine so the gpsimd queue issues its xT
        # DMAs immediately.
        wsp = const.tile([128, 192], BF16)
        nc.vector.memset(wsp, 0.0)

        zbias = const.tile([128, 1], F32)
        nc.vector.memset(zbias, 0.0)
        # exp is the only ACT table set this kernel uses (rsqrt is done with a
        # Newton iteration on DVE); preload it while waiting on input DMAs.
        scratch1 = const.tile([1, 1], F32)
        nc.scalar.activation(scratch1, zbias[0:1, :], AF.Exp, bias=0.0, scale=1.0)

        # pair matrices: p64h[c,g] = 0.25 iff (c%64)//2 == g (both partition
        # halves, each holding one half-image under the rotated-mirror
        # layout); p32x64[g,c] = 1 iff c//2 == g
        p64h = const.tile([128, 32], BF16)
        nc.vector.memset(p64h[0:64, :], 0.25)
        nc.gpsimd.affine_select(out=p64h[0:64, :], in_=p64h[0:64, :],
                                compare_op=ALU.is_ge,
                                fill=0.0, base=0, pattern=[[-2, 32]],
                                channel_multiplier=1)
        nc.gpsimd.affine_select(out=p64h[0:64, :], in_=p64h[0:64, :],
                                compare_op=ALU.is_ge,
                                fill=0.0, base=1, pattern=[[2, 32]],
                                channel_multiplier=-1)
        nc.gpsimd.dma_start(out=p64h[64:128, :], in_=p64h[0:64, :])
        p32x64 = const.tile([32, 64], BF16)
        nc.vector.memset(p32x64, 1.0)
        nc.gpsimd.affine_select(out=p32x64, in_=p32x64, compare_op=ALU.is_ge,
                                fill=0.0, base=0, pattern=[[1, 64]],
                                channel_multiplier=-2)
        nc.gpsimd.affine_select(out=p32x64, in_=p32x64, compare_op=ALU.is_ge,
                                fill=0.0, base=1, pattern=[[-1, 64]],
                                channel_multiplier=2)

        # ---- PSUM pools (8 banks: st 3x[128,1024]=6, acc 2x1) ----
        # Three score buffers break the serial chain exp(p) -> scores(p+2) ->
        # exp(p+2): with bufs=3 a score matmul waits on the exp THREE pairs
        # back (the other engine's, long done), so the two exp engines stream
        # at their native rate.
        st_ps = ctx.enter_context(tc.tile_pool(name="st_ps", bufs=3, space="PSUM"))
        aux_ps = ctx.enter_context(tc.tile_pool(name="aux_ps", bufs=2, space="PSUM"))

        # PE warmup spin (see wsp note above): no-op matmuls bridge the idle
        # window between kernel start and the first real matmuls.
        warm = st_ps.tile([128, 1024], F32, tag="st")
        for _ in range(48):
            nc.tensor.matmul(warm[0:64, 0:128], lhsT=wsp[:, 0:64],
                             rhs=wsp[:, 64:192], start=True, stop=True)

        # ---- GroupNorm stats on DVE: all 128 partitions active; partition
        # ---- p>=64 carries channel p-64 over the second half-image ----
        bstats = tiny.tile([128, 4, 6], F32)
        for i in range(4):
            nc.vector.bn_stats(bstats[:, i, :], xT[:, 512 * i:512 * (i + 1)])
        mv = tiny.tile([128, 2], F32)
        nc.vector.bn_aggr(mv, bstats)
        packed128 = tiny.tile([128, 2], BF16)     # [mean, E[x^2]] per (c,half)
        nc.vector.tensor_copy(packed128[:, 0:1], mv[:, 0:1])
        nc.vector.scalar_tensor_tensor(out=packed128[:, 1:2], in0=mv[:, 0:1],
                                       scalar=mv[:, 0:1], in1=mv[:, 1:2],
                                       op0=ALU.mult, op1=ALU.add)

        # ---- host-packed params (emitted AFTER the stats: the per-queue DMA
        # ---- semaphore waits coalesce in program order, so anything emitted
        # ---- earlier on these rings would gate bn_stats) ----
        pk_sb = const.tile([128, 259], F32)
        nc.scalar.dma_start(out=pk_sb, in_=pk128_d)
        wk_sb = pk_sb[:, 0:64]        # [wk; wk]
        wq_sb = pk_sb[:, 64:128]      # [wq; wq]
        wv_sb = pk_sb[:, 128:192]     # [wv; wv]
        gamma_col = pk_sb[:, 192:193]
        beta_col = pk_sb[0:64, 193:194]
        wo_sb = pk_sb[0:64, 194:258]
        pk65_sb = const.tile([65, 129], F32)
        nc.scalar.dma_start(out=pk65_sb, in_=pk65_d)
        wq_aug = pk65_sb[:, 0:64]     # [Wq ; bq]
        wv_aug = pk65_sb[:, 64:129]   # [Wv ; bv] plus e64 column
        # wo_aug = [wo ; bvo+bo] plus e64 column that passes l through. Row
        # 64 multiplies the l-row of the accumulator, so after the division
        # by l it contributes the constant row (bv_total @ wo) + bo - the v-
        # and output-bias applied without materializing them per-position.
        wo_aug = const.tile([65, 65], BF16)
        nc.scalar.dma_start(out=wo_aug[0:64, :], in_=pkb_d)
        nc.vector.memset(wo_aug[64:65, 64:65], 1.0)
        bo_row = const.tile([1, 64], F32)
        nc.scalar.dma_start(out=bo_row, in_=bo_d.rearrange("(o c) -> o c", o=1))

        gpair = aux_ps.tile([32, 2], F32, tag="aux")  # group [mean, E[x^2]]
        nc.tensor.matmul(gpair, lhsT=p64h, rhs=packed128)
        gm = tiny.tile([32, 2], F32)
        nc.vector.tensor_copy(gm, gpair)
        var = tiny.tile([32, 1], F32)
        nc.vector.tensor_mul(var, gm[:, 0:1], gm[:, 0:1])
        nc.vector.tensor_sub(var, gm[:, 1:2], var)
        nc.vector.tensor_scalar_add(var, var, EPS)
        # rstd = rsqrt(var) entirely on DVE: quake-style bit seed + 1 Newton
        # step (seed err ~3.4% -> ~0.2% after the step; GroupNorm scale is
        # far inside the tolerance) - keeps the scalar engine's activation
        # tables untouched for exp, and the serial chain short.
        U32 = mybir.dt.uint32
        magic = tiny.tile([32, 1], U32)
        nc.vector.memset(magic, 0x5f3759df)
        packed32 = tiny.tile([32, 2], BF16)       # [rstd_g | mean_g]
        nc.vector.tensor_copy(packed32[:, 1:2], gm[:, 0:1])
        ybits = tiny.tile([32, 1], U32)
        nc.vector.tensor_scalar(out=ybits, in0=var.bitcast(U32), scalar1=1,
                                scalar2=None, op0=ALU.logical_shift_right)
        nc.vector.tensor_sub(ybits, magic, ybits)
        y = ybits.bitcast(F32)
        c15 = tiny.tile([32, 1], F32)
        nc.vector.memset(c15, 1.5)
        nhv = tiny.tile([32, 1], F32)
        nc.vector.tensor_scalar_mul(nhv, var, -0.5)
        t1 = tiny.tile([32, 1], F32)
        nc.vector.tensor_mul(t1, y, y)
        nc.vector.scalar_tensor_tensor(out=t1, in0=t1, scalar=nhv, in1=c15,
                                       op0=ALU.mult, op1=ALU.add)
        nc.vector.tensor_mul(packed32[:, 0:1], y, t1)
        rstd = packed32[:, 0:1]
        chan = aux_ps.tile([128, 2], F32, tag="aux")  # expand groups->channels,
        nc.tensor.matmul(chan[0:64, :], lhsT=p32x64, rhs=packed32)  # both halves
        nc.tensor.matmul(chan[64:128, :], lhsT=p32x64, rhs=packed32,
                         tile_position=(0, 64))
        scale_col = tiny.tile([128, 1], F32)      # rstd_g * gamma_c (mirrored)
        nc.vector.tensor_mul(scale_col, chan[:, 0:1], gamma_col)
        gnbias = tiny.tile([65, 1], F32)          # beta - mean*scale, aug 1
        nc.vector.tensor_mul(gnbias[0:64, :], chan[0:64, 1:2], scale_col[0:64, :])
        nc.vector.tensor_sub(gnbias[0:64, :], beta_col, gnbias[0:64, :])
        nc.vector.memset(gnbias[64:65, :], 1.0)

        # ---- fold GN into projection weights (both halves in one op) ----
        wq_sc = tiny.tile([128, 64], BF16)
        nc.vector.tensor_scalar_mul(wq_sc, wq_sb, scale_col)
        wk_sc = tiny.tile([128, 64], BF16)
        nc.vector.tensor_scalar_mul(wk_sc, wk_sb, scale_col)
        wv_sc = tiny.tile([128, 64], BF16)
        nc.vector.tensor_scalar_mul(wv_sc, wv_sb, scale_col)

        bqp = aux_ps.tile([128, 1], F32, tag="aux")  # total q bias, both halves
        nc.tensor.matmul(bqp[0:64, :], lhsT=wq_aug, rhs=gnbias)
        nc.tensor.matmul(bqp[64:128, :], lhsT=wq_aug, rhs=gnbias,
                         tile_position=(0, 64))
        bq_col = tiny.tile([128, 1], F32)
        nc.vector.tensor_copy(bq_col, bqp)
        # bvo row for wo_aug: bvo = (gnbias@Wv + bv) @ wo, bounced through HBM
        # to land on partition 64 (engines are lane-locked; DMA is not). This
        # only gates the first output projection, well off the critical path.
        bvcp = aux_ps.tile([65, 1], F32, tag="aux")
        nc.tensor.matmul(bvcp, lhsT=wv_aug, rhs=gnbias)
        bv_col = tiny.tile([64, 1], F32)
        nc.vector.tensor_copy(bv_col, bvcp[0:64, :])
        bvop = aux_ps.tile([1, 64], F32, tag="aux")
        nc.tensor.matmul(bvop, lhsT=bv_col, rhs=wo_sb)
        # fold the OUTPUT bias bo in as well: row 64 of wo_aug contributes
        # (bvo+bo)*l to the pre-division accumulator, i.e. bvo+bo after the
        # division - so the epilogue residual add uses x directly.
        bvo_row = tiny.tile([1, 64], F32)
        nc.vector.tensor_add(bvo_row, bvop, bo_row)
        bvo_stage = nc.dram_tensor("bvo_stage", [64], F32).ap()
        nc.sync.dma_start(out=bvo_stage.rearrange("(o c) -> o c", o=1), in_=bvo_row)
        nc.gpsimd.dma_start(out=wo_aug[64:65, 0:64],
                            in_=bvo_stage.rearrange("(o c) -> o c", o=1))

        # ---- projections ----
        # Score matmuls pair key-chunk p (array rows 0:63) with chunk 16+p
        # (rows 64:127) - attention is order-invariant over keys - so kT keeps
        # chunks 0:15 on partitions 0:63 and chunks 16:31 on 64:127, and each
        # projection block lands directly on its half (output half chosen by
        # the tile_position column). qT needs every column on BOTH halves:
        # 4 tile-position variants cover the (stripe, half) grid pairwise-
        # concurrently. No SBUF mirror DMAs anywhere.
        kT = big.tile([128, SQ], BF16)
        qT = big.tile([128, SQ], BF16)

        # k/q projected in quad-groups: lo and hi array halves fill the two
        # partition halves of one [128,1024] tile (bank-staggered waves so
        # concurrent row-tiles never drain into the same bank), then ONE
        # full-width copy/bias-add moves both halves. kT's hi half holds
        # chunks 16:31 (see pairing note above); qT needs both halves of
        # every column, which this layout produces naturally.
        def kq_quad(dst, w_sc, lo_cols, hi_cols, bias, on_act):
            g = st_ps.tile([128, 1024], F32, tag="st")
            nc.tensor.matmul(g[0:64, 0:512], lhsT=w_sc[0:64, :],
                             rhs=xT[0:64, lo_cols:lo_cols + 512],
                             tile_position=(0, 0))
            nc.tensor.matmul(g[64:128, 512:1024], lhsT=w_sc[64:128, :],
                             rhs=xT[64:128, hi_cols + 512:hi_cols + 1024],
                             tile_position=(64, 64))
            nc.tensor.matmul(g[0:64, 512:1024], lhsT=w_sc[0:64, :],
                             rhs=xT[0:64, lo_cols + 512:lo_cols + 1024],
                             tile_position=(0, 0))
            nc.tensor.matmul(g[64:128, 0:512], lhsT=w_sc[64:128, :],
                             rhs=xT[64:128, hi_cols:hi_cols + 512],
                             tile_position=(64, 64))
            # drains alternate between ACT and DVE so neither engine builds a
            # pre-loop backlog (ACT applies a per-partition bias for free via
            # Identity+bias AP)
            if bias is None:
                if on_act:
                    nc.scalar.copy(out=dst, in_=g)
                else:
                    nc.vector.tensor_copy(dst, g)
            elif on_act:
                nc.scalar.activation(dst, g, AF.Identity, bias=bias)
            else:
                nc.vector.tensor_scalar_add(dst, g, bias)

        # hi-half column indices are shifted -2048 under the rotated mirror
        kq_quad(kT[:, 0:1024], wk_sc, 0, 0, None, True)
        kq_quad(qT[:, 0:1024], wq_sc, 0, 2048, bq_col, True)
        kq_quad(kT[:, 1024:2048], wk_sc, 1024, 1024, None, False)
        kq_quad(qT[:, 1024:2048], wq_sc, 1024, 3072, bq_col, False)

        # v in natural [t, c] layout; groups of 4 chunks {p, 8+p, 16+p, 24+p}
        # share one PSUM bank (quarter slices) and drain with one strided copy.
        # Chunks p,8+p ride array rows 0:63, 16+p,24+p rows 64:127 so the two
        # sub-pairs run concurrently. Column 64 = ones via one strided memset.
        v_big = big.tile([128, N_CHUNK, 65], BF16)
        nc.vector.memset(v_big[:, :, 64:65], 1.0)
        # view: chunk (a*16 + b*8 + p) -> [p_, a, b, p, c]; lo-rows compute the
        # a=0 chunks into bank 0 of a 2-bank tile, hi-rows the a=1 chunks into
        # bank 1 (concurrent row-tiles must drain into distinct banks).
        v4 = v_big.rearrange("q (a b g) c -> q a b g c", a=2, b=2)

        def v_group(p):
            vga = aux_ps.tile([128, 2, 64], F32, tag="aux")
            vgb = aux_ps.tile([128, 2, 64], F32, tag="aux")
            for a, vg in ((0, vga), (1, vgb)):
                half = slice(64, 128) if a else slice(0, 64)
                tp = (64, 0) if a else (0, 0)
                for b in range(2):
                    ch = a * 16 + b * 8 + p
                    # hi-half (a=1) columns shift -2048 (rotated mirror)
                    c0 = 128 * ch - 2048 * a
                    nc.tensor.matmul(vg[:, b, :],
                                     lhsT=xT[half, c0:c0 + 128],
                                     rhs=wv_sc[half, :], tile_position=tp)
            # drains split across ACT/DVE so neither builds a pre-loop backlog
            nc.scalar.copy(out=v4[:, 0, :, p, 0:64], in_=vga)
            nc.vector.tensor_copy(v4[:, 1, :, p, 0:64], vgb)

        for p in range(8):
            v_group(p)

        # ---- residual rows (needed only by epilogues; bo is folded into the
        # ---- wo_aug l-row, so x is used directly) ----
        xq_sb = big.tile([128, NQ, 64], F32)
        xq_r = x_q.rearrange("(m p) c -> p m c", p=128)
        nc.sync.dma_start(out=xq_sb[:, 0:8, :], in_=xq_r[:, 0:8, :])
        nc.scalar.dma_start(out=xq_sb[:, 8:16, :], in_=xq_r[:, 8:16, :])

        # ---- main attention loop ----
        # Pairs of key chunks: the two K=64 score matmuls run concurrently on
        # the two row-halves of the PE array into the two banks of one PSUM
        # tile; exp covers both in one instruction, alternating between the
        # Scalar engine's LUT exp and the Vector engine's bit-trick exp so the
        # two engines stream concurrently. att@v for chunk c is ONE full-K=128
        # matmul accumulating into a single PSUM bank (no merge needed), and
        # trails the score/exp stream by LAG pairs: the PE issues in order, so
        # by the time it reaches att@v(p), exp(p) - issued LAG pairs earlier -
        # is already complete and the queue never stalls.
        p_pool = ctx.enter_context(tc.tile_pool(name="p_pool", bufs=6))
        ep_pool = ctx.enter_context(tc.tile_pool(name="ep_pool", bufs=3))
        N_PAIR = N_CHUNK // 2
        LAG = 3
        # 7 odd pairs on DVE, 9 on ACT: balances ACT's 1.11us/tile + drain-lo
        # vs DVE's 1.22us/tile + drain-hi + epilogue work.
        DVE_PAIRS = {1, 3, 5, 7, 9, 11, 13}

        def emit_o(io, ot_lo, ot_hi, pt, first, last):
            # K=64 halves on opposite array row-halves run concurrently and
            # let LDWEIGHTS overlap in-flight matmuls (different row groups);
            # each half accumulates into its own bank. The banks are never
            # summed directly: the output projection is linear, so the
            # epilogue projects each half separately and accumulates in PSUM.
            nc.tensor.matmul(ot_lo, lhsT=v_big[0:64, io, :], rhs=pt[0:64, :],
                             tile_position=(0, 0), start=first, stop=last)
            nc.tensor.matmul(ot_hi, lhsT=v_big[64:128, io, :], rhs=pt[64:128, :],
                             tile_position=(64, 0), start=first, stop=last)

        def make_epilogue(j, lo_sb, hi_sb, tail=False):
            def epi():
                res = ep_pool.tile([128, 4, 64], F32, tag="res", bufs=2)
                rl = ep_pool.tile([128, 4], F32, tag="rl")
                # borrows an st-pool slot mid-stripe; the slot's previous
                # score tile is consumed by then and its next user comes ~3
                # pairs later, so the rotation never stalls.
                opq = st_ps.tile([128, 4, 65], F32, tag="st")
                for m in range(4):
                    cs = slice(128 * m, 128 * (m + 1))
                    nc.tensor.matmul(opq[:, m, :], lhsT=lo_sb[:, cs],
                                     rhs=wo_aug, start=True, stop=False)
                    nc.tensor.matmul(opq[:, m, :], lhsT=hi_sb[:, cs],
                                     rhs=wo_aug, start=False, stop=True)
                if tail:
                    # the last stripe is the critical exit path: normalize and
                    # ship each 128-row chunk on its own DMA ring so the
                    # ~650ns per-issue cost doesn't serialize
                    for m in range(4):
                        nc.vector.reciprocal(rl[:, m:m + 1], opq[:, m, 64:65])
                        nc.vector.scalar_tensor_tensor(
                            out=res[:, m, :], in0=opq[:, m, 0:64],
                            scalar=rl[:, m:m + 1], in1=xq_sb[:, 4 * j + m, :],
                            op0=ALU.mult, op1=ALU.add)
                        base = 512 * j + 128 * m
                        ring = [nc.sync, nc.scalar, nc.gpsimd, nc.sync][m]
                        ring.dma_start(out=out_d[base:base + 128, :],
                                       in_=res[:, m, :])
                else:
                    nc.vector.reciprocal(rl, opq[:, :, 64])
                    for m in range(4):
                        nc.vector.scalar_tensor_tensor(
                            out=res[:, m, :], in0=opq[:, m, 0:64],
                            scalar=rl[:, m:m + 1], in1=xq_sb[:, 4 * j + m, :],
                            op0=ALU.mult, op1=ALU.add)
                    nc.sync.dma_start(
                        out=out_d[512 * j:512 * (j + 1), :].rearrange("(m p) c -> p m c", p=128),
                        in_=res)
            return epi

        # Flat 64-pair pipeline: pair g = stripe g//16, pair-in-stripe g%16.
        # The att@v stream trails by LAG pairs and crosses stripe boundaries
        # without draining the pipe; each stripe's accumulator drains are
        # emitted right after its last att@v (landing ~2 pairs into the next
        # stripe on the exp engines' queues), and its epilogue 3 pairs after
        # that, so neither ever blocks the next stripe's exps.
        NG = N_STRIPE * N_PAIR
        pts = {}
        ots = {}
        lohi = {}
        for g in range(NG + LAG):
            if g < NG:
                j, p = divmod(g, N_PAIR)
                if p == 0:
                    ots[j] = (aux_ps.tile([65, 512], F32, tag="aux", name=f"ot_lo{j}"),
                              aux_ps.tile([65, 512], F32, tag="aux", name=f"ot_hi{j}"))
                st2 = st_ps.tile([128, 1024], F32, tag="st")
                nc.tensor.matmul(st2[:, 0:512],
                                 lhsT=kT[0:64, 128 * p:128 * (p + 1)],
                                 rhs=qT[0:64, 512 * j:512 * (j + 1)],
                                 tile_position=(0, 0))
                nc.tensor.matmul(st2[:, 512:1024],
                                 lhsT=kT[64:128, 128 * p:128 * (p + 1)],
                                 rhs=qT[64:128, 512 * j:512 * (j + 1)],
                                 tile_position=(64, 0))
                pt = p_pool.tile([128, 1024], BF16, tag="p")
                if p in DVE_PAIRS:
                    nc.vector.tensor_scalar(pt.bitcast(U16), st2, EXP_A,
                                            EXP_B, ALU.mult, ALU.add)
                else:
                    nc.scalar.activation(pt, st2, AF.Exp, bias=zbias,
                                         scale=SCALE)
                pts[g] = pt
            if g >= LAG:
                go = g - LAG
                jo, po = divmod(go, N_PAIR)
                pt = pts.pop(go)
                ot_lo, ot_hi = ots[jo]
                emit_o(po, ot_lo, ot_hi, pt[:, 0:512], po == 0, False)
                emit_o(16 + po, ot_lo, ot_hi, pt[:, 512:1024], False,
                       po == N_PAIR - 1)
                if po == N_PAIR - 1:
                    # drain the stripe's two accumulator banks (lo on Scalar,
                    # hi on Vector so neither becomes the bottleneck)
                    lo_sb = ep_pool.tile([65, 512], BF16, bufs=2, tag="lo_sb")
                    hi_sb = ep_pool.tile([65, 512], BF16, bufs=2, tag="hi_sb")
                    nc.scalar.copy(out=lo_sb, in_=ot_lo)
                    nc.vector.tensor_copy(hi_sb, ot_hi)
                    lohi[jo] = (lo_sb, hi_sb)
                    del ots[jo]
                if po == 2 and jo >= 1:
                    make_epilogue(jo - 1, *lohi.pop(jo - 1))()
        make_epilogue(N_STRIPE - 1, *lohi.pop(N_STRIPE - 1), tail=True)()


_NC_CACHE = {}


def _get_nc():
    if "nc" not in _NC_CACHE:
        _NC_CACHE["nc"] = build_kernel()
    return _NC_CACHE["nc"]


def build_in_maps(x, gamma, beta, wq, bq, wk, wv, bv, wo, bo):
    """Per-core NEFF input dicts plus (batch, rows) scatter info per core."""
    x = np.asarray(x, dtype=np.float32)
    gamma = np.asarray(gamma, np.float32)
    beta = np.asarray(beta, np.float32)
    wq = np.asarray(wq, np.float32)
    bq = np.asarray(bq, np.float32)
    wk = np.asarray(wk, np.float32)
    wv = np.asarray(wv, np.float32)
    bv = np.asarray(bv, np.float32)
    wo = np.asarray(wo, np.float32)
    bo = np.asarray(bo, np.float32)
    # pack the small weights into three layout blocks (see _emit)
    pk128 = np.zeros((128, 259), np.float32)
    pk128[0:64, 0:64] = wk
    pk128[64:128, 0:64] = wk
    pk128[0:64, 64:128] = wq
    pk128[64:128, 64:128] = wq
    pk128[0:64, 128:192] = wv
    pk128[64:128, 128:192] = wv
    pk128[0:64, 192] = gamma
    pk128[64:128, 192] = gamma
    pk128[0:64, 193] = beta
    pk128[0:64, 194:258] = wo
    pk65 = np.zeros((65, 129), np.float32)
    pk65[0:64, 0:64] = wq
    pk65[64, 0:64] = bq
    pk65[0:64, 64:128] = wv
    pk65[64, 64:128] = bv
    pk65[64, 128] = 1.0
    pkb = np.zeros((64, 65), np.float32)
    pkb[:, 0:64] = wo
    shared = {
        "pk128": pk128,
        "pk65": pk65,
        "pkb": pkb.astype(ml_dtypes.bfloat16),
        "bo": bo,
    }
    xf = x.reshape(B, S, C)
    in_maps = []
    scatter = []
    for core in range(8):
        b, h = core // 2, core % 2
        own = slice(h * SQ, (h + 1) * SQ)
        other = slice((1 - h) * SQ, (2 - h) * SQ)
        x_local = np.concatenate([xf[b][own], xf[b][other]], axis=0)
        # partitions 0:64 = channels x positions; 64:128 = the same rotated
        # by 2048 columns (see the kernel's rotated-mirror layout note)
        xt = np.ascontiguousarray(x_local.T).astype(ml_dtypes.bfloat16)
        xt2 = np.concatenate([xt, np.roll(xt, -SQ, axis=1)], axis=0)
        in_maps.append({
            "xT": np.ascontiguousarray(xt2),
            "x_q": np.ascontiguousarray(x_local[:SQ]),
            **shared,
        })
        scatter.append((b, np.arange(h * SQ, (h + 1) * SQ)))
    return in_maps, scatter


def _run(in_maps, scatter, **spmd_kwargs):
    nc = _get_nc()
    res = run_bass_kernel_spmd(nc, in_maps, core_ids=list(range(8)),
                               **spmd_kwargs)
    out = np.empty((B, S, C), np.float32)
    for core in range(8):
        b, rows = scatter[core]
        out[b][rows] = res.results[core]["out"]
    return out.reshape(B, H, W, C), res


def kernel(x, gamma, beta, wq, bq, wk, bk, wv, bv, wo, bo):
    # bk is provably a no-op: it shifts each query's scores by the constant
    # bk.q which softmax cancels, so it is not shipped to the device.
    in_maps, scatter = build_in_maps(x, gamma, beta, wq, bq, wk, wv, bv, wo, bo)
    out, _ = _run(in_maps, scatter)
    return out



# revision 72
# speedup vs baseline: 1.2226x; 1.0364x over previous
"""Trainium2 Bass kernel for nn_AttentionBlock (B=4, H=W=64, C=64, GROUPS=32).

Math (reference):
    hn = GroupNorm(x; gamma, beta, 32 groups, eps=1e-3)
    q = hn@wq+bq ; k = hn@wk+bk ; v = hn@wv+bv
    att = softmax(q k^T / 8) over the 4096 spatial positions
    out = x + (att @ v) @ wo + bo

Sharding: data-parallel, 2 cores per batch image, each core owns 2048 of the
4096 queries but holds the full key/value set for its batch. No collectives.

Per-core pipeline (fully fused on one NeuronCore):
  - xT [C=64, S=4096] arrives pre-transposed in bf16 (host does the cheap
    numpy transpose+cast), so channel-contraction matmuls need no on-chip
    transposes. x_q keeps the core's own query rows in fp32 for the residual.
  - GroupNorm stats via bn_stats/bn_aggr per channel on DVE, then tiny 0/1
    matmuls pair-combine channels into groups and expand back. The GN affine
    folds into the projection weights: W~ = diag(scale_c)@W, b~ = gnbias@W + b.
  - k-bias is dropped: it shifts each query's scores by a constant, which
    softmax cancels exactly.
  - Scores are computed transposed, ST[t, s] (keys on partitions), so exp(ST)
    feeds the att@v matmul directly as the moving operand - the attention
    matrix is never transposed. Score matmuls have K=64, so two key-chunks run
    CONCURRENTLY on the two halves of the PE array (row-tiling): chunk p rides
    rows 0:63 and chunk 16+p rides rows 64:127 (kT stores each group on its
    own partition half; qT carries every column on both halves).
  - Softmax is max-free: |score| <= ~3 for unit-normal inputs so exp cannot
    overflow, and softmax(x) == softmax(x - max) exactly.
  - exp() runs one ACT instruction per chunk-pair over a 2-bank PSUM tile to
    amortize the ~352-cycle activation pipeline latency.
  - v gets an appended ones-column so att@v also accumulates the softmax
    denominator l[s]. att@v is split into two K=64 halves accumulating into
    two PSUM banks (summed by one DVE add at stripe end): the halves run on
    opposite array halves, letting LDWEIGHTS overlap in-flight matmuls.
  - The output projection runs on the unnormalized accumulator ((O/l)@wo ==
    (O@wo)/l), with an extra wo column passing l through; one reciprocal +
    fused multiply-add applies softmax normalization, residual and bo.
"""

import numpy as np
import ml_dtypes

import concourse.tile as tile
from concourse import bacc, mybir
from concourse.bass_utils import run_bass_kernel_spmd

F32 = mybir.dt.float32
BF16 = mybir.dt.bfloat16
U16 = mybir.dt.uint16
AF = mybir.ActivationFunctionType
ALU = mybir.AluOpType

# DVE bit-trick exp (Schraudolph in bf16): u16 = (uint16)(s*EXP_A + EXP_B);
# reinterpreting those bits as bf16 gives exp(s/8) with <=3.4% rel error
# (mean +0.2%), which softmax-averaging shrinks well below the tolerance.
# This lets the Vector engine absorb ~44% of the exp stream that otherwise
# serializes on the Scalar engine's activation LUT (1 elem/cycle/lane).
EXP_A = 0.125 * 1.4426950408889634 * 128.0   # scale * log2(e) * 2^7
EXP_B = 16256.0 - 5.5                        # 127*2^7 - c  (c tuned numerically)

B, H, W, C = 4, 64, 64, 64
S = H * W            # 4096 spatial positions per image
SQ = S // 2          # 2048 queries per core
EPS = 1e-3
N_CHUNK = S // 128   # 32 key chunks
NQ = SQ // 128       # 16 query chunks
N_STRIPE = SQ // 512  # 4 query stripes
SCALE = float(C) ** -0.5  # 0.125


def build_kernel():
    nc = bacc.Bacc("TRN2", target_bir_lowering=False, debug=False)

    xT_d = nc.dram_tensor("xT", [2 * C, S], BF16, kind="ExternalInput")
    x_q = nc.dram_tensor("x_q", [SQ, C], F32, kind="ExternalInput")
    # host-packed parameter blocks (pure layout prep: stacking/casting the
    # tiny 1x1-conv weights) - one DMA each instead of ~18 small DMAs whose
    # serial ~650ns descriptor-issue cost used to gate the whole preamble
    pk128 = nc.dram_tensor("pk128", [128, 259], F32, kind="ExternalInput")
    pk65 = nc.dram_tensor("pk65", [65, 129], F32, kind="ExternalInput")
    pkb = nc.dram_tensor("pkb", [C, 65], BF16, kind="ExternalInput")
    bo_d = nc.dram_tensor("bo", [C], F32, kind="ExternalInput")
    out_d = nc.dram_tensor("out", [SQ, C], F32, kind="ExternalOutput")

    with tile.TileContext(nc) as tc:
        _emit(nc, tc, xT_d.ap(), x_q.ap(), pk128.ap(), pk65.ap(), pkb.ap(),
              bo_d.ap(), out_d.ap())
    nc.compile()
    return nc


def _emit(nc, tc, xT_d, x_q, pk128_d, pk65_d, pkb_d, bo_d, out_d):
    from contextlib import ExitStack

    ctx = ExitStack()
    with ctx:
        const = ctx.enter_context(tc.tile_pool(name="const", bufs=1))
        big = ctx.enter_context(tc.tile_pool(name="big", bufs=1))
        tiny = ctx.enter_context(tc.tile_pool(name="tiny", bufs=1))

        # ---- big input DMAs first (sync/HWDGE ring), chunked so dependents
        # ---- can start early. Partitions 64:127 hold a copy ROTATED by 2048
        # ---- columns (xT[64:128, c] = x[:, (c+2048) % S]): row-tiled matmuls
        # ---- just index hi-half columns with a -2048 shift, and GroupNorm
        # ---- stats get a [128, 2048] view covering all 4096 positions with
        # ---- every partition lane active (2.4x faster bn_stats).
        # The host ships BOTH partition halves (hi half pre-rotated), so the
        # four chunk DMAs are independent - no SBUF->SBUF mirror chain.
        xT = big.tile([128, S], BF16)
        eng = [nc.sync, nc.scalar, nc.gpsimd, nc.sync,
               nc.scalar, nc.gpsimd, nc.sync, nc.scalar]
        for i in range(8):
            eng[i].dma_start(out=xT[:, 512 * i:512 * (i + 1)],
                             in_=xT_d[:, 512 * i:512 * (i + 1)])

        # ---- PE warmup spin: the PE clock sits at 1.2 GHz until ~3.4us of
        # sustained activity un-throttles it to 2.4 GHz. Dummy matmuls on a
        # zeroed tile during the DMA/stats wait warm it up so the projections
        # and the first attention stripe run at full rate. Memsets run on the
        # (otherwise idle) Vector engine so the gpsimd queue issues its xT
        # DMAs immediately.
        wsp = const.tile([128, 192], BF16)
        nc.vector.memset(wsp, 0.0)

        zbias = const.tile([128, 1], F32)
        nc.vector.memset(zbias, 0.0)
        # exp is the only ACT table set this kernel uses (rsqrt is done with a
        # Newton iteration on DVE); preload it while waiting on input DMAs.
        scratch1 = const.tile([1, 1], F32)
        nc.scalar.activation(scratch1, zbias[0:1, :], AF.Exp, bias=0.0, scale=1.0)

        # pair matrices: p64h[c,g] = 0.25 iff (c%64)//2 == g (both partition
        # halves, each holding one half-image under the rotated-mirror
        # layout); p32x64[g,c] = 1 iff c//2 == g
        p64h = const.tile([128, 32], BF16)
        nc.vector.memset(p64h[0:64, :], 0.25)
        nc.gpsimd.affine_select(out=p64h[0:64, :], in_=p64h[0:64, :],
                                compare_op=ALU.is_ge,
                                fill=0.0, base=0, pattern=[[-2, 32]],
                                channel_multiplier=1)
        nc.gpsimd.affine_select(out=p64h[0:64, :], in_=p64h[0:64, :],
                                compare_op=ALU.is_ge,
                                fill=0.0, base=1, pattern=[[2, 32]],
                                channel_multiplier=-1)
        nc.gpsimd.dma_start(out=p64h[64:128, :], in_=p64h[0:64, :])
        p32x64 = const.tile([32, 64], BF16)
        nc.vector.memset(p32x64, 1.0)
        nc.gpsimd.affine_select(out=p32x64, in_=p32x64, compare_op=ALU.is_ge,
                                fill=0.0, base=0, pattern=[[1, 64]],
                                channel_multiplier=-2)
        nc.gpsimd.affine_select(out=p32x64, in_=p32x64, compare_op=ALU.is_ge,
                                fill=0.0, base=1, pattern=[[-1, 64]],
                                channel_multiplier=2)

        # ---- PSUM pools (8 banks: st 3x[128,1024]=6, acc 2x1) ----
        # Three score buffers break the serial chain exp(p) -> scores(p+2) ->
        # exp(p+2): with bufs=3 a score matmul waits on the exp THREE pairs
        # back (the other engine's, long done), so the two exp engines stream
        # at their native rate.
        st_ps = ctx.enter_context(tc.tile_pool(name="st_ps", bufs=3, space="PSUM"))
        aux_ps = ctx.enter_context(tc.tile_pool(name="aux_ps", bufs=2, space="PSUM"))

        # PE warmup spin (see wsp note above): no-op matmuls bridge the idle
        # window between kernel start and the first real matmuls.
        warm = st_ps.tile([128, 1024], F32, tag="st")
        for _ in range(48):
            nc.tensor.matmul(warm[0:64, 0:128], lhsT=wsp[:, 0:64],
                             rhs=wsp[:, 64:192], start=True, stop=True)

        # ---- GroupNorm stats on DVE: all 128 partitions active; partition
        # ---- p>=64 carries channel p-64 over the second half-image ----
        bstats = tiny.tile([128, 4, 6], F32)
        for i in range(4):
            nc.vector.bn_stats(bstats[:, i, :], xT[:, 512 * i:512 * (i + 1)])
        mv = tiny.tile([128, 2], F32)
        nc.vector.bn_aggr(mv, bstats)
        packed128 = tiny.tile([128, 2], BF16)     # [mean, E[x^2]] per (c,half)
        nc.vector.tensor_copy(packed128[:, 0:1], mv[:, 0:1])
        nc.vector.scalar_tensor_tensor(out=packed128[:, 1:2], in0=mv[:, 0:1],
                                       scalar=mv[:, 0:1], in1=mv[:, 1:2],
                                       op0=ALU.mult, op1=ALU.add)

        # ---- host-packed params (emitted AFTER the stats: the per-queue DMA
        # ---- semaphore waits coalesce in program order, so anything emitted
        # ---- earlier on these rings would gate bn_stats) ----
        pk_sb = const.tile([128, 259], F32)
        nc.scalar.dma_start(out=pk_sb, in_=pk128_d)
        wk_sb = pk_sb[:, 0:64]        # [wk; wk]
        wq_sb = pk_sb[:, 64:128]      # [wq; wq]
        wv_sb = pk_sb[:, 128:192]     # [wv; wv]
        gamma_col = pk_sb[:, 192:193]
        beta_col = pk_sb[0:64, 193:194]
        wo_sb = pk_sb[0:64, 194:258]
        pk65_sb = const.tile([65, 129], F32)
        nc.scalar.dma_start(out=pk65_sb, in_=pk65_d)
        wq_aug = pk65_sb[:, 0:64]     # [Wq ; bq]
        wv_aug = pk65_sb[:, 64:129]   # [Wv ; bv] plus e64 column
        # wo_aug = [wo ; bvo+bo] plus e64 column that passes l through. Row
        # 64 multiplies the l-row of the accumulator, so after the division
        # by l it contributes the constant row (bv_total @ wo) + bo - the v-
        # and output-bias applied without materializing them per-position.
        wo_aug = const.tile([65, 65], BF16)
        nc.scalar.dma_start(out=wo_aug[0:64, :], in_=pkb_d)
        nc.vector.memset(wo_aug[64:65, 64:65], 1.0)
        bo_row = const.tile([1, 64], F32)
        nc.scalar.dma_start(out=bo_row, in_=bo_d.rearrange("(o c) -> o c", o=1))

        gpair = aux_ps.tile([32, 2], F32, tag="aux")  # group [mean, E[x^2]]
        nc.tensor.matmul(gpair, lhsT=p64h, rhs=packed128)
        gm = tiny.tile([32, 2], F32)
        nc.vector.tensor_copy(gm, gpair)
        var = tiny.tile([32, 1], F32)
        nc.vector.tensor_mul(var, gm[:, 0:1], gm[:, 0:1])
        nc.vector.tensor_sub(var, gm[:, 1:2], var)
        nc.vector.tensor_scalar_add(var, var, EPS)
        # rstd = rsqrt(var) entirely on DVE: quake-style bit seed + 1 Newton
        # step (seed err ~3.4% -> ~0.2% after the step; GroupNorm scale is
        # far inside the tolerance) - keeps the scalar engine's activation
        # tables untouched for exp, and the serial chain short.
        U32 = mybir.dt.uint32
        magic = tiny.tile([32, 1], U32)
        nc.vector.memset(magic, 0x5f3759df)
        packed32 = tiny.tile([32, 2], BF16)       # [rstd_g | mean_g]
        nc.vector.tensor_copy(packed32[:, 1:2], gm[:, 0:1])
        ybits = tiny.tile([32, 1], U32)
        nc.vector.tensor_scalar(out=ybits, in0=var.bitcast(U32), scalar1=1,
                                scalar2=None, op0=ALU.logical_shift_right)
        nc.vector.tensor_sub(ybits, magic, ybits)
        y = ybits.bitcast(F32)
        c15 = tiny.tile([32, 1], F32)
        nc.vector.memset(c15, 1.5)
        nhv = tiny.tile([32, 1], F32)
        nc.vector.tensor_scalar_mul(nhv, var, -0.5)
        t1 = tiny.tile([32, 1], F32)
        nc.vector.tensor_mul(t1, y, y)
        nc.vector.scalar_tensor_tensor(out=t1, in0=t1, scalar=nhv, in1=c15,
                                       op0=ALU.mult, op1=ALU.add)
        nc.vector.tensor_mul(packed32[:, 0:1], y, t1)
        rstd = packed32[:, 0:1]
        chan = aux_ps.tile([128, 2], F32, tag="aux")  # expand groups->channels,
        nc.tensor.matmul(chan[0:64, :], lhsT=p32x64, rhs=packed32)  # both halves
        nc.tensor.matmul(chan[64:128, :], lhsT=p32x64, rhs=packed32,
                         tile_position=(0, 64))
        scale_col = tiny.tile([128, 1], F32)      # rstd_g * gamma_c (mirrored)
        nc.vector.tensor_mul(scale_col, chan[:, 0:1], gamma_col)
        gnbias = tiny.tile([65, 1], F32)          # beta - mean*scale, aug 1
        nc.vector.tensor_mul(gnbias[0:64, :], chan[0:64, 1:2], scale_col[0:64, :])
        nc.vector.tensor_sub(gnbias[0:64, :], beta_col, gnbias[0:64, :])
        nc.vector.memset(gnbias[64:65, :], 1.0)

        # ---- fold GN into projection weights (both halves in one op) ----
        wq_sc = tiny.tile([128, 64], BF16)
        nc.vector.tensor_scalar_mul(wq_sc, wq_sb, scale_col)
        wk_sc = tiny.tile([128, 64], BF16)
        nc.vector.tensor_scalar_mul(wk_sc, wk_sb, scale_col)
        wv_sc = tiny.tile([128, 64], BF16)
        nc.vector.tensor_scalar_mul(wv_sc, wv_sb, scale_col)

        bqp = aux_ps.tile([128, 1], F32, tag="aux")  # total q bias, both halves
        nc.tensor.matmul(bqp[0:64, :], lhsT=wq_aug, rhs=gnbias)
        nc.tensor.matmul(bqp[64:128, :], lhsT=wq_aug, rhs=gnbias,
                         tile_position=(0, 64))
        bq_col = tiny.tile([128, 1], F32)
        nc.vector.tensor_copy(bq_col, bqp)
        # bvo row for wo_aug: bvo = (gnbias@Wv + bv) @ wo, bounced through HBM
        # to land on partition 64 (engines are lane-locked; DMA is not). This
        # only gates the first output projection, well off the critical path.
        bvcp = aux_ps.tile([65, 1], F32, tag="aux")
        nc.tensor.matmul(bvcp, lhsT=wv_aug, rhs=gnbias)
        bv_col = tiny.tile([64, 1], F32)
        nc.vector.tensor_copy(bv_col, bvcp[0:64, :])
        bvop = aux_ps.tile([1, 64], F32, tag="aux")
        nc.tensor.matmul(bvop, lhsT=bv_col, rhs=wo_sb)
        # fold the OUTPUT bias bo in as well: row 64 of wo_aug contributes
        # (bvo+bo)*l to the pre-division accumulator, i.e. bvo+bo after the
        # division - so the epilogue residual add uses x directly.
        bvo_row = tiny.tile([1, 64], F32)
        nc.vector.tensor_add(bvo_row, bvop, bo_row)
        bvo_stage = nc.dram_tensor("bvo_stage", [64], F32).ap()
        nc.sync.dma_start(out=bvo_stage.rearrange("(o c) -> o c", o=1), in_=bvo_row)
        nc.gpsimd.dma_start(out=wo_aug[64:65, 0:64],
                            in_=bvo_stage.rearrange("(o c) -> o c", o=1))

        # ---- projections ----
        # Score matmuls pair key-chunk p (array rows 0:63) with chunk 16+p
        # (rows 64:127) - attention is order-invariant over keys - so kT keeps
        # chunks 0:15 on partitions 0:63 and chunks 16:31 on 64:127, and each
        # projection block lands directly on its half (output half chosen by
        # the tile_position column). qT needs every column on BOTH halves:
        # 4 tile-position variants cover the (stripe, half) grid pairwise-
        # concurrently. No SBUF mirror DMAs anywhere.
        kT = big.tile([128, SQ], BF16)
        qT = big.tile([128, SQ], BF16)

        # k/q projected in quad-groups: lo and hi array halves fill the two
        # partition halves of one [128,1024] tile (bank-staggered waves so
        # concurrent row-tiles never drain into the same bank), then ONE
        # full-width copy/bias-add moves both halves. kT's hi half holds
        # chunks 16:31 (see pairing note above); qT needs both halves of
        # every column, which this layout produces naturally.
        def kq_quad(dst, w_sc, lo_cols, hi_cols, bias, on_act):
            g = st_ps.tile([128, 1024], F32, tag="st")
            nc.tensor.matmul(g[0:64, 0:512], lhsT=w_sc[0:64, :],
                             rhs=xT[0:64, lo_cols:lo_cols + 512],
                             tile_position=(0, 0))
            nc.tensor.matmul(g[64:128, 512:1024], lhsT=w_sc[64:128, :],
                             rhs=xT[64:128, hi_cols + 512:hi_cols + 1024],
                             tile_position=(64, 64))
            nc.tensor.matmul(g[0:64, 512:1024], lhsT=w_sc[0:64, :],
                             rhs=xT[0:64, lo_cols + 512:lo_cols + 1024],
                             tile_position=(0, 0))
            nc.tensor.matmul(g[64:128, 0:512], lhsT=w_sc[64:128, :],
                             rhs=xT[64:128, hi_cols:hi_cols + 512],
                             tile_position=(64, 64))
            # drains alternate between ACT and DVE so neither engine builds a
            # pre-loop backlog (ACT applies a per-partition bias for free via
            # Identity+bias AP)
            if bias is None:
                if on_act:
                    nc.scalar.copy(out=dst, in_=g)
                else:
                    nc.vector.tensor_copy(dst, g)
            elif on_act:
                nc.scalar.activation(dst, g, AF.Identity, bias=bias)
            else:
                nc.vector.tensor_scalar_add(dst, g, bias)

        # hi-half column indices are shifted -2048 under the rotated mirror
        kq_quad(kT[:, 0:1024], wk_sc, 0, 0, None, True)
        kq_quad(qT[:, 0:1024], wq_sc, 0, 2048, bq_col, True)
        kq_quad(kT[:, 1024:2048], wk_sc, 1024, 1024, None, False)
        kq_quad(qT[:, 1024:2048], wq_sc, 1024, 3072, bq_col, False)

        # v in natural [t, c] layout; groups of 4 chunks {p, 8+p, 16+p, 24+p}
        # share one PSUM bank (quarter slices) and drain with one strided copy.
        # Chunks p,8+p ride array rows 0:63, 16+p,24+p rows 64:127 so the two
        # sub-pairs run concurrently. Column 64 = ones via one strided memset.
        v_big = big.tile([128, N_CHUNK, 65], BF16)
        nc.vector.memset(v_big[:, :, 64:65], 1.0)
        # view: chunk (a*16 + b*8 + p) -> [p_, a, b, p, c]; lo-rows compute the
        # a=0 chunks into bank 0 of a 2-bank tile, hi-rows the a=1 chunks into
        # bank 1 (concurrent row-tiles must drain into distinct banks).
        v4 = v_big.rearrange("q (a b g) c -> q a b g c", a=2, b=2)

        def v_group(p):
            vga = aux_ps.tile([128, 2, 64], F32, tag="aux")
            vgb = aux_ps.tile([128, 2, 64], F32, tag="aux")
            for a, vg in ((0, vga), (1, vgb)):
                half = slice(64, 128) if a else slice(0, 64)
                tp = (64, 0) if a else (0, 0)
                for b in range(2):
                    ch = a * 16 + b * 8 + p
                    # hi-half (a=1) columns shift -2048 (rotated mirror)
                    c0 = 128 * ch - 2048 * a
                    nc.tensor.matmul(vg[:, b, :],
                                     lhsT=xT[half, c0:c0 + 128],
                                     rhs=wv_sc[half, :], tile_position=tp)
            # drains split across ACT/DVE so neither builds a pre-loop backlog
            nc.scalar.copy(out=v4[:, 0, :, p, 0:64], in_=vga)
            nc.vector.tensor_copy(v4[:, 1, :, p, 0:64], vgb)

        for p in range(8):
            v_group(p)

        # ---- residual rows (needed only by epilogues; bo is folded into the
        # ---- wo_aug l-row, so x is used directly) ----
        xq_sb = big.tile([128, NQ, 64], F32)
        xq_r = x_q.rearrange("(m p) c -> p m c", p=128)
        nc.sync.dma_start(out=xq_sb[:, 0:8, :], in_=xq_r[:, 0:8, :])
        nc.scalar.dma_start(out=xq_sb[:, 8:16, :], in_=xq_r[:, 8:16, :])

        # ---- main attention loop ----
        # Pairs of key chunks: the two K=64 score matmuls run concurrently on
        # the two row-halves of the PE array into the two banks of one PSUM
        # tile; exp covers both in one instruction, alternating between the
        # Scalar engine's LUT exp and the Vector engine's bit-trick exp so the
        # two engines stream concurrently. att@v for chunk c is ONE full-K=128
        # matmul accumulating into a single PSUM bank (no merge needed), and
        # trails the score/exp stream by LAG pairs: the PE issues in order, so
        # by the time it reaches att@v(p), exp(p) - issued LAG pairs earlier -
        # is already complete and the queue never stalls.
        p_pool = ctx.enter_context(tc.tile_pool(name="p_pool", bufs=6))
        ep_pool = ctx.enter_context(tc.tile_pool(name="ep_pool", bufs=3))
        N_PAIR = N_CHUNK // 2
        LAG = 3
        # 7 odd pairs on DVE, 9 on ACT: balances ACT's 1.11us/tile + drain-lo
        # vs DVE's 1.22us/tile + drain-hi + epilogue work.
        DVE_PAIRS = {1, 3, 5, 7, 9, 11, 13}

        def emit_o(io, ot_lo, ot_hi, pt, first, last):
            # K=64 halves on opposite array row-halves run concurrently and
            # let LDWEIGHTS overlap in-flight matmuls (different row groups);
            # each half accumulates into its own bank. The banks are never
            # summed directly: the output projection is linear, so the
            # epilogue projects each half separately and accumulates in PSUM.
            nc.tensor.matmul(ot_lo, lhsT=v_big[0:64, io, :], rhs=pt[0:64, :],
                             tile_position=(0, 0), start=first, stop=last)
            nc.tensor.matmul(ot_hi, lhsT=v_big[64:128, io, :], rhs=pt[64:128, :],
                             tile_position=(64, 0), start=first, stop=last)

        def make_epilogue(j, lo_sb, hi_sb, tail=False):
            def epi():
                res = ep_pool.tile([128, 4, 64], F32, tag="res", bufs=2)
                rl = ep_pool.tile([128, 4], F32, tag="rl")
                # borrows an st-pool slot mid-stripe; the slot's previous
                # score tile is consumed by then and its next user comes ~3
                # pairs later, so the rotation never stalls.
                opq = st_ps.tile([128, 4, 65], F32, tag="st")
                for m in range(4):
                    cs = slice(128 * m, 128 * (m + 1))
                    nc.tensor.matmul(opq[:, m, :], lhsT=lo_sb[:, cs],
                                     rhs=wo_aug, start=True, stop=False)
                    nc.tensor.matmul(opq[:, m, :], lhsT=hi_sb[:, cs],
                                     rhs=wo_aug, start=False, stop=True)
                if tail:
                    # the last stripe is the critical exit path: normalize and
                    # ship each 128-row chunk on its own DMA ring so the
                    # ~650ns per-issue cost doesn't serialize
                    for m in range(4):
                        nc.vector.reciprocal(rl[:, m:m + 1], opq[:, m, 64:65])
                        nc.vector.scalar_tensor_tensor(
                            out=res[:, m, :], in0=opq[:, m, 0:64],
                            scalar=rl[:, m:m + 1], in1=xq_sb[:, 4 * j + m, :],
                            op0=ALU.mult, op1=ALU.add)
                        base = 512 * j + 128 * m
                        ring = [nc.sync, nc.scalar, nc.gpsimd, nc.sync][m]
                        ring.dma_start(out=out_d[base:base + 128, :],
                                       in_=res[:, m, :])
                else:
                    nc.vector.reciprocal(rl, opq[:, :, 64])
                    for m in range(4):
                        nc.vector.scalar_tensor_tensor(
                            out=res[:, m, :], in0=opq[:, m, 0:64],
                            scalar=rl[:, m:m + 1], in1=xq_sb[:, 4 * j + m, :],
                            op0=ALU.mult, op1=ALU.add)
                    nc.sync.dma_start(
                        out=out_d[512 * j:512 * (j + 1), :].rearrange("(m p) c -> p m c", p=128),
                        in_=res)
            return epi

        pending_epilogue = None
        for j in range(N_STRIPE):
            last_stripe = j == N_STRIPE - 1
            ot_lo = aux_ps.tile([65, 512], F32, tag="aux")
            ot_hi = aux_ps.tile([65, 512], F32, tag="aux")
            pts = {}
            for p in range(N_PAIR + LAG):
                if p < N_PAIR:
                    st2 = st_ps.tile([128, 1024], F32, tag="st")
                    nc.tensor.matmul(st2[:, 0:512],
                                     lhsT=kT[0:64, 128 * p:128 * (p + 1)],
                                     rhs=qT[0:64, 512 * j:512 * (j + 1)],
                                     tile_position=(0, 0))
                    nc.tensor.matmul(st2[:, 512:1024],
                                     lhsT=kT[64:128, 128 * p:128 * (p + 1)],
                                     rhs=qT[64:128, 512 * j:512 * (j + 1)],
                                     tile_position=(64, 0))
                    pt = p_pool.tile([128, 1024], BF16, tag="p")
                    if p in DVE_PAIRS:
                        nc.vector.tensor_scalar(pt.bitcast(U16), st2, EXP_A,
                                                EXP_B, ALU.mult, ALU.add)
                    else:
                        nc.scalar.activation(pt, st2, AF.Exp, bias=zbias,
                                             scale=SCALE)
                    pts[p] = pt
                if p == 15 and pending_epilogue is not None:
                    pending_epilogue()
                    pending_epilogue = None
                po = p - LAG
                if po >= 0:
                    pt = pts.pop(po)
                    emit_o(po, ot_lo, ot_hi, pt[:, 0:512], po == 0, False)
                    emit_o(16 + po, ot_lo, ot_hi, pt[:, 512:1024], False,
                           po == N_PAIR - 1)
            # drain the two accumulator banks to SBUF for the projection (lo
            # on Scalar, hi on Vector so neither engine becomes the
            # bottleneck). The last stripe drains per column-chunk and runs
            # its epilogue inline - it is the critical tail.
            lo_sb = ep_pool.tile([65, 512], BF16, bufs=2, tag="lo_sb")
            hi_sb = ep_pool.tile([65, 512], BF16, bufs=2, tag="hi_sb")
            if last_stripe:
                res = ep_pool.tile([128, 4, 64], F32, tag="res", bufs=2)
                rl = ep_pool.tile([128, 4], F32, tag="rl")
                opq = st_ps.tile([128, 4, 65], F32, tag="st")
                for m in range(4):
                    cs = slice(128 * m, 128 * (m + 1))
                    nc.scalar.copy(out=lo_sb[:, cs], in_=ot_lo[:, cs])
                    nc.vector.tensor_copy(hi_sb[:, cs], ot_hi[:, cs])
                    nc.tensor.matmul(opq[:, m, :], lhsT=lo_sb[:, cs],
                                     rhs=wo_aug, start=True, stop=False)
                    nc.tensor.matmul(opq[:, m, :], lhsT=hi_sb[:, cs],
                                     rhs=wo_aug, start=False, stop=True)
                    nc.vector.reciprocal(rl[:, m:m + 1], opq[:, m, 64:65])
                    nc.vector.scalar_tensor_tensor(out=res[:, m, :],
                                                   in0=opq[:, m, 0:64],
                                                   scalar=rl[:, m:m + 1],
                                                   in1=xq_sb[:, 4 * j + m, :],
                                                   op0=ALU.mult, op1=ALU.add)
                    # tail latency: ship each chunk on its own ring so the
                    # ~650ns per-issue cost doesn't serialize
                    base = 512 * j + 128 * m
                    ring = [nc.sync, nc.scalar, nc.gpsimd, nc.sync][m]
                    ring.dma_start(out=out_d[base:base + 128, :],
                                   in_=res[:, m, :])
            else:
                nc.scalar.copy(out=lo_sb, in_=ot_lo)
                nc.vector.tensor_copy(hi_sb, ot_hi)
                pending_epilogue = make_epilogue(j, lo_sb, hi_sb)


_NC_CACHE = {}


def _get_nc():
    if "nc" not in _NC_CACHE:
        _NC_CACHE["nc"] = build_kernel()
    return _NC_CACHE["nc"]


def build_in_maps(x, gamma, beta, wq, bq, wk, wv, bv, wo, bo):
    """Per-core NEFF input dicts plus (batch, rows) scatter info per core."""
    x = np.asarray(x, dtype=np.float32)
    gamma = np.asarray(gamma, np.float32)
    beta = np.asarray(beta, np.float32)
    wq = np.asarray(wq, np.float32)
    bq = np.asarray(bq, np.float32)
    wk = np.asarray(wk, np.float32)
    wv = np.asarray(wv, np.float32)
    bv = np.asarray(bv, np.float32)
    wo = np.asarray(wo, np.float32)
    bo = np.asarray(bo, np.float32)
    # pack the small weights into three layout blocks (see _emit)
    pk128 = np.zeros((128, 259), np.float32)
    pk128[0:64, 0:64] = wk
    pk128[64:128, 0:64] = wk
    pk128[0:64, 64:128] = wq
    pk128[64:128, 64:128] = wq
    pk128[0:64, 128:192] = wv
    pk128[64:128, 128:192] = wv
    pk128[0:64, 192] = gamma
    pk128[64:128, 192] = gamma
    pk128[0:64, 193] = beta
    pk128[0:64, 194:258] = wo
    pk65 = np.zeros((65, 129), np.float32)
    pk65[0:64, 0:64] = wq
    pk65[64, 0:64] = bq
    pk65[0:64, 64:128] = wv
    pk65[64, 64:128] = bv
    pk65[64, 128] = 1.0
    pkb = np.zeros((64, 65), np.float32)
    pkb[:, 0:64] = wo
    shared = {
        "pk128": pk128,
        "pk65": pk65,
        "pkb": pkb.astype(ml_dtypes.bfloat16),
        "bo": bo,
    }
    xf = x.reshape(B, S, C)
    in_maps = []
    scatter = []
    for core in range(8):
        b, h = core // 2, core % 2
        own = slice(h * SQ, (h + 1) * SQ)
        other = slice((1 - h) * SQ, (2 - h) * SQ)
        x_local = np.concatenate([xf[b][own], xf[b][other]], axis=0)
        # partitions 0:64 = channels x positions; 64:128 = the same rotated
        # by 2048 columns (see the kernel's rotated-mirror layout note)
        xt = np.ascontiguousarray(x_local.T).astype(ml_dtypes.bfloat16)
        xt2 = np.concatenate([xt, np.roll(xt, -SQ, axis=1)], axis=0)
        in_maps.append({
            "xT": np.ascontiguousarray(xt2),
            "x_q": np.ascontiguousarray(x_local[:SQ]),
            **shared,
        })
        scatter.append((b, np.arange(h * SQ, (h + 1) * SQ)))
    return in_maps, scatter


def _run(in_maps, scatter, **spmd_kwargs):
    nc = _get_nc()
    res = run_bass_kernel_spmd(nc, in_maps, core_ids=list(range(8)),
                               **spmd_kwargs)
    out = np.empty((B, S, C), np.float32)
    for core in range(8):
        b, rows = scatter[core]
        out[b][rows] = res.results[core]["out"]
    return out.reshape(B, H, W, C), res


def kernel(x, gamma, beta, wq, bq, wk, bk, wv, bv, wo, bo):
    # bk is provably a no-op: it shifts each query's scores by the constant
    # bk.q which softmax cancels, so it is not shipped to the device.
    in_maps, scatter = build_in_maps(x, gamma, beta, wq, bq, wk, wv, bv, wo, bo)
    out, _ = _run(in_maps, scatter)
    return out

